# revision 7
# baseline (speedup 1.0000x reference)
"""GQA causal self-attention with RoPE for Trainium2 — single-core version.

Problem: B=4, T=2048, C=1024, H=16 q-heads, Hkv=4 kv-heads, D=64, fp32.

Why single-core: in this environment the per-call wall time is dominated by
the PJRT-tunnel round trip plus a per-(argument x device) dispatch overhead
(~0.4ms per extra buffer binding per device, ~2.5ms per extra device).
Device compute for the whole problem is ~2.1ms, far below the ~28-70ms
tunnel floor. So the fastest configuration is ONE core with the fewest
possible buffer arguments: everything (x, weights, RoPE tables, the RoPE
permutation matrix) packed into ONE bf16 blob, one f32 output — two buffer
bindings total. (8-core sharding costs ~+20ms/call in pure dispatch
overhead; a separate f32 table blob costs ~+0.4ms.)

Math identical to the 8-core baseline, serialized over (batch, head-group):
  - group g of batch b: q-heads pair p -> (8g+p, 8g+p+4), kv heads (2g,2g+1)
  - projections accumulate over C in PSUM; RoPE = two DVE mults by cs/sn
    rows + PE matmul by a 32-block-swap permutation + DVE add
  - v is PE-transposed with a fused ones column so PV emits the softmax
    denominator for free (PSUM row 64)
  - logits are bounded (scale-0.02 weights), so exp without max-subtraction,
    bf16, causal masking via precomputed 0/1 masks on DVE, diagonal blocks
    column-trimmed
  - normalize: DVE reciprocal of the denom row + PE broadcast matmul
    (ones[1,64]^T @ recip[1,512] -> PSUM[64,512]) + DVE multiply.  This
    replaces the baseline's GPSIMD DRAM-bounce broadcast (slow SWDGE).
  - out-proj accumulates both head groups' contributions in PSUM (8
    matmuls per [128,512] tile), so no host-side partial sums.

bf16 is used for x, all weights, q^T/k^T, and y^T: logits here are tiny
(|s|/8 ~ 0.2) so softmax is insensitive to 0.5% quantization, and the two
1024-term contractions average the elementwise error down. Measured rel
err vs the f32 reference stays ~5e-3 (limit 2e-2).
"""

import numpy as np

B, T, C = 4, 2048, 1024
H, HKV, D = 16, 4, 64
NCORES = 1
DQ = 512      # q cols per group (8 heads x 64)
DKV = 128     # kv cols per group (2 heads x 64)
ROPE_THETA = 10000.0

TT = 512      # query tile
SB = 128      # key block
NTT = T // TT         # 4
NSB_ALL = T // SB     # 16
KO = C // 128         # 8

# --- packed input layouts (columns) ---
XCOLS = KO * T                 # 16384 per batch, layout [128, ko, T]
X0 = 0                         # 4 batches
WQ0 = X0 + B * XCOLS           # 65536; per g: [128, ko, 512] -> 4096 cols
WK0 = WQ0 + 2 * KO * DQ        # 73728; per g: [128, ko, 128] -> 1024 cols
WV0 = WK0 + 2 * KO * DKV       # 75776
WO0 = WV0 + 2 * KO * DKV       # 77824; per g: [128, pair, 1024] -> 4096 cols
CS0 = WO0 + 2 * 4 * C          # 86016
SN0 = CS0 + T                  # 88064
PM0 = SN0 + T                  # 90112
N16 = PM0 + 128                # 90240

last_results = None
_timing_state = None


def _build_nc():
    import concourse.bass as bass
    import concourse.mybir as mybir
    import concourse.tile as tile
    from concourse import bacc
    from contextlib import ExitStack

    F32 = mybir.dt.float32
    F32R = mybir.dt.float32r
    BF16 = mybir.dt.bfloat16
    Exp = mybir.ActivationFunctionType.Exp
    ts = bass.ts

    nc = bacc.Bacc("TRN2", target_bir_lowering=False, debug=False,
                   num_devices=NCORES)

    b16 = nc.dram_tensor("b16", [128, N16], BF16, kind="ExternalInput")
    out = nc.dram_tensor("out", [B * T, C], F32, kind="ExternalOutput")

    with tile.TileContext(nc) as tc, ExitStack() as big:
        per = big.enter_context(tc.tile_pool(name="per", bufs=1))
        cs_sb = per.tile([128, T], BF16)
        sn_sb = per.tile([128, T], BF16)
        pm_sb = per.tile([128, 128], BF16)
        id_sb = per.tile([128, 128], F32)
        ones_sb = per.tile([128, 64], F32)
        mk_sb = per.tile([128, 4, 1024], BF16)
        wq_sb = per.tile([128, 2, KO * DQ], BF16)
        wk_sb = per.tile([128, 2, KO * DKV], BF16)
        wv_sb = per.tile([128, 2, KO * DKV], BF16)
        wo_sb = per.tile([128, 2, 4 * C], BF16)

        nc.sync.dma_start(cs_sb[:], b16[:, CS0:CS0 + T])
        nc.sync.dma_start(sn_sb[:], b16[:, SN0:SN0 + T])
        nc.sync.dma_start(pm_sb[:], b16[:, PM0:PM0 + 128])
        for g in range(2):
            nc.sync.dma_start(wq_sb[:, g, :], b16[:, WQ0 + g * KO * DQ:
                                                   WQ0 + (g + 1) * KO * DQ])
            nc.sync.dma_start(wk_sb[:, g, :], b16[:, WK0 + g * KO * DKV:
                                                   WK0 + (g + 1) * KO * DKV])
            nc.sync.dma_start(wv_sb[:, g, :], b16[:, WV0 + g * KO * DKV:
                                                   WV0 + (g + 1) * KO * DKV])
            nc.sync.dma_start(wo_sb[:, g, :], b16[:, WO0 + g * 4 * C:
                                                   WO0 + (g + 1) * 4 * C])

        from concourse.masks import make_identity
        make_identity(nc, id_sb[:])
        nc.vector.memset(ones_sb[:], 1.0)
        # mk[k][x, y] = 1 if (y % 512) >= 128*k + x else 0  (doubled [a|b])
        for k in range(4):
            for half in range(2):
                m = mk_sb[:, k, half * 512:(half + 1) * 512]
                nc.vector.memset(m, 1.0)
                nc.gpsimd.affine_select(
                    out=m, in_=m,
                    compare_op=mybir.AluOpType.is_ge,
                    fill=0.0, base=-128 * k,
                    pattern=[[1, 512]],
                    channel_multiplier=-1,
                )

        xpool = big.enter_context(tc.tile_pool(name="xpool", bufs=2))
        vpool = big.enter_context(tc.tile_pool(name="vpool", bufs=2))
        atmp = big.enter_context(tc.tile_pool(name="atmp", bufs=2))
        gpool = big.enter_context(tc.tile_pool(name="gpool", bufs=2))
        ypool = big.enter_context(tc.tile_pool(name="ypool", bufs=2))
        ppool = big.enter_context(tc.tile_pool(name="ppool", bufs=6))
        npool = big.enter_context(tc.tile_pool(name="npool", bufs=2))
        obuf = big.enter_context(tc.tile_pool(name="obuf", bufs=3))

        for b in range(B):
            qts, kts, vas = [], [], []
            # ---- phase A: projections + RoPE + v transpose (both groups) --
            with ExitStack() as pa:
                apsum = pa.enter_context(
                    tc.tile_pool(name="apsum", bufs=4, space="PSUM"))
                rpsum = pa.enter_context(
                    tc.tile_pool(name="rpsum", bufs=2, space="PSUM"))
                tpsum = pa.enter_context(
                    tc.tile_pool(name="tpsum", bufs=2, space="PSUM"))

                def rope_to(ps, dst, c0):
                    """ps: [128,TT] psum pre-RoPE rows -> rope'd into dst."""
                    ta = atmp.tile([128, TT], F32, tag="ta")
                    tb = atmp.tile([128, TT], BF16, tag="tb")
                    nc.vector.tensor_mul(ta[:], ps, cs_sb[:, c0:c0 + TT])
                    nc.vector.tensor_mul(tb[:], ps, sn_sb[:, c0:c0 + TT])
                    pr = rpsum.tile([128, TT], F32, tag="pr")
                    nc.tensor.matmul(pr[:], lhsT=pm_sb[:], rhs=tb[:],
                                     start=True, stop=True)
                    nc.vector.tensor_add(dst, ta[:], pr[:])

                for g in range(2):
                    qt = gpool.tile([128, 4 * T], BF16, tag="qt")
                    kt = gpool.tile([128, T], BF16, tag="kt")
                    va = gpool.tile([128, NSB_ALL, 130], BF16, tag="va")
                    qts.append(qt); kts.append(kt); vas.append(va)
                    nc.vector.memset(va[:, :, 64:65], 1.0)
                    nc.vector.memset(va[:, :, 129:130], 1.0)
                for tt in range(NTT):
                    xt = xpool.tile([128, KO * TT], BF16, tag="xt")
                    for ko in range(KO):
                        nc.sync.dma_start(
                            xt[:, ts(ko, TT)],
                            b16[:, X0 + b * XCOLS + ko * T + tt * TT:
                                X0 + b * XCOLS + ko * T + (tt + 1) * TT])
                    for g in range(2):
                        qt, kt, va = qts[g], kts[g], vas[g]
                        for p in range(4):
                            ps = apsum.tile([128, TT], F32, tag="pq")
                            for ko in range(KO):
                                nc.tensor.matmul(
                                    ps[:],
                                    lhsT=wq_sb[:, g, ko * DQ + p * 128:
                                               ko * DQ + (p + 1) * 128],
                                    rhs=xt[:, ts(ko, TT)],
                                    start=(ko == 0), stop=(ko == KO - 1))
                            rope_to(ps[:], qt[:, p * T + tt * TT:
                                              p * T + (tt + 1) * TT], tt * TT)
                        ps = apsum.tile([128, TT], F32, tag="pq")
                        for ko in range(KO):
                            nc.tensor.matmul(
                                ps[:],
                                lhsT=wk_sb[:, g, ts(ko, DKV)],
                                rhs=xt[:, ts(ko, TT)],
                                start=(ko == 0), stop=(ko == KO - 1))
                        rope_to(ps[:], kt[:, ts(tt, TT)], tt * TT)
                        ps = apsum.tile([128, TT], F32, tag="pq")
                        for ko in range(KO):
                            nc.tensor.matmul(
                                ps[:],
                                lhsT=wv_sb[:, g, ts(ko, DKV)],
                                rhs=xt[:, ts(ko, TT)],
                                start=(ko == 0), stop=(ko == KO - 1))
                        vt = vpool.tile([128, TT], F32, tag="vt")
                        nc.scalar.copy(vt[:], ps[:])
                        for j in range(TT // 128):
                            sb = tt * (TT // 128) + j
                            pt = tpsum.tile([128, 128], F32, tag="pt")
                            nc.tensor.transpose(pt[:], vt[:, ts(j, 128)],
                                                id_sb[:])
                            nc.scalar.copy(va[:, sb, 0:64], pt[:, 0:64])
                            nc.scalar.copy(va[:, sb, 65:129], pt[:, 64:128])

            # ---- phase B: attention (both groups) ----
            yts = []
            with ExitStack() as pb:
                spsum = pb.enter_context(
                    tc.tile_pool(name="spsum", bufs=2, space="PSUM"))
                # pv accumulators double-buffered so the in-order PE can
                # start the next (tt,p) PV chain as soon as the previous
                # tile's reciprocal+copy have read it; the broadcast psum
                # tiles rotate through the same slots (their WAR on those
                # reads is already implied by dataflow), so no extra banks.
                vpsum = pb.enter_context(
                    tc.tile_pool(name="vpsum", bufs=2, space="PSUM"))
                for g in range(2):
                    qt, kt, va = qts[g], kts[g], vas[g]
                    yt = ypool.tile([128, 4 * T], BF16, tag="yt")
                    yts.append(yt)
                    for tt in range(NTT):
                        for p in range(4):
                            nsb = 4 * tt + 4
                            pv_a = vpsum.tile([128, TT], F32, tag="pva")
                            pv_b = vpsum.tile([128, TT], F32, tag="pvb")
                            for sb in range(nsb):
                                k = sb - 4 * tt
                                coff = 0 if k < 1 else 128 * k
                                eoff = 0 if k < 1 else 128 * k
                                st = spsum.tile([128, 1024], F32, tag="st")
                                nc.tensor.matmul(
                                    st[:, coff:512],
                                    lhsT=kt[0:64, ts(sb, SB)],
                                    rhs=qt[0:64, p * T + tt * TT + coff:
                                           p * T + (tt + 1) * TT],
                                    start=True, stop=True)
                                nc.tensor.matmul(
                                    st[:, 512 + coff:1024],
                                    lhsT=kt[64:128, ts(sb, SB)],
                                    rhs=qt[64:128, p * T + tt * TT + coff:
                                           p * T + (tt + 1) * TT],
                                    start=True, stop=True)
                                pe = ppool.tile([128, 1024], BF16, tag="pe")
                                if eoff == 0:
                                    nc.scalar.activation(pe[:], st[:], Exp,
                                                         scale=0.125)
                                else:
                                    nc.scalar.activation(
                                        pe[:, eoff:512], st[:, eoff:512],
                                        Exp, scale=0.125)
                                    nc.scalar.activation(
                                        pe[:, 512 + eoff:1024],
                                        st[:, 512 + eoff:1024],
                                        Exp, scale=0.125)
                                if k == 0:
                                    nc.vector.tensor_mul(pe[:], pe[:],
                                                         mk_sb[:, k, :])
                                elif k >= 1:
                                    nc.vector.tensor_mul(
                                        pe[:, eoff:512], pe[:, eoff:512],
                                        mk_sb[:, k, eoff:512])
                                    nc.vector.tensor_mul(
                                        pe[:, 512 + eoff:1024],
                                        pe[:, 512 + eoff:1024],
                                        mk_sb[:, k, eoff:512])
                                # PV trimmed to causally-live query columns:
                                # queries < eoff see nothing in block k, and
                                # the sb==0 matmul (always full-width) opens
                                # the accumulation for every column.
                                nc.tensor.matmul(
                                    pv_a[0:65, eoff:512],
                                    lhsT=va[:, sb, 0:65],
                                    rhs=pe[:, eoff:512],
                                    start=(sb == 0), stop=(sb == nsb - 1))
                                nc.tensor.matmul(
                                    pv_b[0:65, eoff:512],
                                    lhsT=va[:, sb, 65:130],
                                    rhs=pe[:, 512 + eoff:1024],
                                    start=(sb == 0), stop=(sb == nsb - 1))
                            # normalize: reciprocal + PE broadcast + multiply
                            # (y rows copied to SBUF first: DVE can read at
                            # most one PSUM operand per instruction)
                            ycol = p * T + tt * TT
                            ra = npool.tile([128, TT], F32, tag="r")
                            nc.vector.reciprocal(ra[64:65, :], pv_a[64:65, :])
                            ya = npool.tile([128, TT], F32, tag="yc")
                            nc.scalar.copy(ya[0:64, :], pv_a[0:64, :])
                            bca = vpsum.tile([128, TT], F32, tag="pva")
                            nc.tensor.matmul(bca[0:64, :],
                                             lhsT=ones_sb[64:65, 0:64],
                                             rhs=ra[64:65, :],
                                             start=True, stop=True)
                            nc.vector.tensor_mul(
                                yt[0:64, ycol:ycol + TT],
                                ya[0:64, :], bca[0:64, :])
                            rb = npool.tile([128, TT], F32, tag="r")
                            nc.vector.reciprocal(rb[64:65, :], pv_b[64:65, :])
                            yb0 = npool.tile([128, TT], F32, tag="yc")
                            nc.scalar.copy(yb0[0:64, :], pv_b[0:64, :])
                            bcb = vpsum.tile([128, TT], F32, tag="pvb")
                            nc.tensor.matmul(bcb[0:64, :],
                                             lhsT=ones_sb[64:65, 0:64],
                                             rhs=rb[64:65, :],
                                             start=True, stop=True)
                            yb = npool.tile([128, TT], BF16, tag="yb")
                            nc.vector.tensor_mul(yb[0:64, :], yb0[0:64, :],
                                                 bcb[0:64, :])
                            nc.sync.dma_start(yt[64:128, ycol:ycol + TT],
                                              yb[0:64, :])

            # ---- phase C: output projection (groups accumulated) ----
            with ExitStack() as pc:
                opsum = pc.enter_context(
                    tc.tile_pool(name="opsum", bufs=4, space="PSUM"))
                for t8 in range(T // 128):
                    for ct in range(C // 512):
                        po = opsum.tile([128, 512], F32, tag="po")
                        idx = 0
                        for g in range(2):
                            for p in range(4):
                                nc.tensor.matmul(
                                    po[:],
                                    lhsT=yts[g][:, p * T + t8 * 128:
                                                p * T + (t8 + 1) * 128],
                                    rhs=wo_sb[:, g, p * C + ct * 512:
                                              p * C + (ct + 1) * 512],
                                    start=(idx == 0), stop=(idx == 7))
                                idx += 1
                        so = obuf.tile([128, 512], F32, tag="so")
                        nc.vector.tensor_copy(so[:], po[:])
                        nc.sync.dma_start(
                            out[b * T + t8 * 128: b * T + (t8 + 1) * 128,
                                ts(ct, 512)], so[:])

    nc.finalize()
    return nc


def _rope_tables():
    invf = (1.0 / (ROPE_THETA ** (np.arange(0, D, 2, dtype=np.float32) / D))
            ).astype(np.float32)
    t = np.arange(T, dtype=np.float32)
    fr = np.outer(t, invf).astype(np.float32)          # [T, 32]
    cosv = np.cos(fr).astype(np.float32).T             # [32, T]
    sinv = np.sin(fr).astype(np.float32).T
    cs = np.tile(cosv, (4, 1))                         # [128, T]
    sn = np.concatenate([sinv, -sinv, sinv, -sinv], axis=0)
    return np.ascontiguousarray(cs), np.ascontiguousarray(sn)


def _perm_matrix():
    pmat = np.zeros((128, 128), dtype=np.float32)
    for m in range(128):
        sig = m + 32 if (m // 32) % 2 == 0 else m - 32
        pmat[sig, m] = 1.0
    return pmat


def _perm_cols_qk(heads):
    idx = []
    for h in heads:
        idx.extend(h * D + np.arange(0, D, 2))
        idx.extend(h * D + np.arange(1, D, 2))
    return np.array(idx, dtype=np.int64)


def _p128(a, blocks, width):
    """[blocks*128, width] -> [128, blocks*width] (partition-major blocks)."""
    return np.ascontiguousarray(
        a.reshape(blocks, 128, width).transpose(1, 0, 2).reshape(
            128, blocks * width))


def _pack_inputs(x, Wq, Wk, Wv, Wo, np_bf16):
    cols16 = []
    for b in range(B):
        cols16.append(_p128(np.ascontiguousarray(x[b].T), KO, T))
    for g in range(2):
        qheads = []
        for p in range(4):
            qheads.extend([8 * g + p, 8 * g + p + 4])
        cols16.append(_p128(Wq[:, _perm_cols_qk(qheads)], KO, DQ))
    for g in range(2):
        kvheads = [2 * g, 2 * g + 1]
        cols16.append(_p128(Wk[:, _perm_cols_qk(kvheads)], KO, DKV))
    for g in range(2):
        kvheads = [2 * g, 2 * g + 1]
        vcols = np.concatenate([kv * D + np.arange(D) for kv in kvheads])
        cols16.append(_p128(Wv[:, vcols], KO, DKV))
    for g in range(2):
        qheads = []
        for p in range(4):
            qheads.extend([8 * g + p, 8 * g + p + 4])
        orows = np.concatenate([h * D + np.arange(D) for h in qheads])
        cols16.append(_p128(Wo[orows, :], 4, C))
    cs, sn = _rope_tables()
    cols16.append(cs)
    cols16.append(sn)
    cols16.append(_perm_matrix())
    blob16 = np.concatenate(cols16, axis=1).astype(np_bf16)
    assert blob16.shape == (128, N16)
    return blob16


def kernel(x, Wq, Wk, Wv, Wo):
    global last_results, _timing_state
    import concourse.mybir as mybir
    from concourse.bass_utils import run_bass_kernel_spmd

    x = np.asarray(x, dtype=np.float32)
    Wq = np.asarray(Wq, dtype=np.float32)
    Wk = np.asarray(Wk, dtype=np.float32)
    Wv = np.asarray(Wv, dtype=np.float32)
    Wo = np.asarray(Wo, dtype=np.float32)

    np_bf16 = mybir.dt.np(mybir.dt.bfloat16)
    blob16 = _pack_inputs(x, Wq, Wk, Wv, Wo, np_bf16)
    nc = _build_nc()

    in_maps = [{"b16": blob16}]
    res = run_bass_kernel_spmd(nc, in_maps, core_ids=[0])
    last_results = res
    _timing_state = (nc, in_maps)

    return np.ascontiguousarray(
        res.results[0]["out"].reshape(B, T, H * D).astype(np.float32))


def time_runs(n=20):
    """Re-execute the last kernel via a cached PJRT callable; min wall ns."""
    import time as _time
    if _timing_state is None:
        return None
    nc, in_maps = _timing_state
    try:
        import jax
        import concourse.mybir as mybir
        from concourse import bass2jax

        bass2jax.install_neuronx_cc_hook()
        partition_name = (
            nc.partition_id_tensor.name if nc.partition_id_tensor else None
        )
        in_names, out_names, out_avals, zero_outs = [], [], [], []
        for alloc in nc.m.functions[0].allocations:
            if not isinstance(alloc, mybir.MemoryLocationSet):
                continue
            name = alloc.memorylocations[0].name
            if alloc.kind == "ExternalInput":
                if name != partition_name:
                    in_names.append(name)
            elif alloc.kind == "ExternalOutput":
                shape = tuple(alloc.tensor_shape)
                dtype = mybir.dt.np(alloc.dtype)
                out_names.append(name)
                out_avals.append(jax.core.ShapedArray(shape, dtype))
                zero_outs.append(np.zeros(shape, dtype))
        all_in_names = list(in_names) + out_names
        if partition_name is not None:
            all_in_names.append(partition_name)

        def _body(*args):
            operands = list(args)
            if partition_name is not None:
                operands.append(bass2jax.partition_id_tensor())
            return tuple(bass2jax._bass_exec_p.bind(
                *operands,
                out_avals=tuple(out_avals),
                in_names=tuple(all_in_names),
                out_names=tuple(out_names),
                lowering_input_output_aliases=(),
                sim_require_finite=True,
                sim_require_nnan=True,
                nc=nc,
            ))

        dev = jax.devices()[0]
        jitted = jax.jit(_body, keep_unused=True)
        args = [jax.device_put(in_maps[0][nm], dev) for nm in in_names]
        args += [jax.device_put(z, dev) for z in zero_outs]
        r = jitted(*args)
        jax.block_until_ready(r)
        best = float("inf")
        for _ in range(n):
            t0 = _time.perf_counter()
            r = jitted(*args)
            jax.block_until_ready(r)
            best = min(best, _time.perf_counter() - t0)
        return best * 1e9
    except Exception:
        import traceback
        traceback.print_exc()
        return None
